# revision 9
# baseline (speedup 1.0000x reference)
"""Block-sparse matmul kernel for Trainium2 (8 NeuronCores, SPMD).

Problem: out = relu(x @ W_sparse + bias)
  x      [1024, 4096] f32
  kernel [4096, 32, 32] f32   (active 32x32 blocks)
  bias   [4096] f32
  ci, co [4096] int32         (block-row / block-col of each active block)
  out    [1024, 4096] f32

Strategy (v3, block-sparse + PE 32x32 array tiling + per-core branches):
2 batch-halves x 4 co-quarters across 8 cores (core = h*4 + q).  Each
block is one 32x32 matmul on PE sub-array (r=ci%4, c=strip of its
column): rhs = x k-tile slice [32, 512], stationary = block weights.
The per-quarter matmul streams differ in their baked x offsets, so the
Tensor instruction stream branches on partition_id via tc.If (4 blocks
per wave); everything else (DMA, eviction) is shared.  Each output
column gets a [32, 512] psum region per row-group r (different PE row
tiles must not share a PSUM bank), 4 columns per wave x 8 waves, with
wave-parity double buffering across the two 4-bank "quads".  Eviction:
DVE strided tensor_reduce sums the 4 row-group partials, ScalarE relu
+ bf16 downcast, DMA out.  Bias is folded in as one extra weight block
per column (row 0 = bias slice, rhs = ones).  Weight loads dominate
the Tensor timeline (~30ns serialized LDWEIGHTS per 32-col weight
tile), so total MM count is the main cost driver.
"""

import numpy as np
import ml_dtypes

import concourse.bacc as bacc
import concourse.bass as bass
import concourse.mybir as mybir
import concourse.tile as tile
from concourse.bass_utils import run_bass_kernel_spmd

BS = 32
N_IN = 4096
N_OUT = 4096
BATCH = 1024
N_CORES = 8

N_COLS = N_OUT // BS          # 128 block-cols
N_KT = N_IN // 128            # 32 k-tiles
N_Q = 4                       # co quarters
COLS_PER_Q = N_COLS // N_Q    # 32
B_PER_CORE = BATCH // 2       # 512 batch rows per core (half)
N_WAVES = 8
N_STRIPS = 4

BF16 = mybir.dt.bfloat16
F32 = mybir.dt.float32

_CACHE = {}


# ----------------------------------------------------------------------
# schedule construction
# ----------------------------------------------------------------------

def _build_schedule(ci, co):
    """Partition cols into 4 quarters, assign each quarter's 32 cols to
    (wave, strip) slots, build per-(q, w, r, c) slot lists padded to the
    cross-quarter max size, and pack weights per strip.

    slot = (kt, kind, val): kind in {"blk","bias","pad"}; wcol implicit
    (sequential per strip in emission order, identical across quarters).
    """
    ci = np.asarray(ci).astype(np.int64)
    co = np.asarray(co).astype(np.int64)

    cols = [[] for _ in range(N_COLS)]
    for n in range(len(ci)):
        cols[co[n]].append((int(ci[n] % 4), int(ci[n] // 4), n))
    njr = np.zeros((N_COLS, 4), np.int64)
    for j in range(N_COLS):
        for (r, _, _) in cols[j]:
            njr[j, r] += 1

    # cols -> quarters (balance totals, capacity 32 each)
    order = np.argsort(-njr.sum(1), kind="stable")
    qtot = np.zeros(N_Q, np.int64)
    qcnt = np.zeros(N_Q, np.int64)
    colq = np.zeros(N_COLS, np.int64)
    for j in order:
        qs = [q for q in range(N_Q) if qcnt[q] < COLS_PER_Q]
        q = min(qs, key=lambda q: qtot[q])
        colq[j] = q
        qcnt[q] += 1
        qtot[q] += njr[j].sum()

    # per quarter: 32 cols -> (wave, strip) slots, one col per slot
    colmap = [None] * N_COLS          # j -> (q, w, c)
    qslots = []                       # [q][w][r][c] -> list of slots
    for q in range(N_Q):
        js = [j for j in range(N_COLS) if colq[j] == q]
        js.sort(key=lambda j: -njr[j].sum())
        load = np.zeros((N_WAVES, 4, N_STRIPS), np.int64)
        used = np.zeros((N_WAVES, N_STRIPS), bool)
        slots = [[[[] for _ in range(N_STRIPS)] for _ in range(4)]
                 for _ in range(N_WAVES)]
        for j in js:
            best, bkey = None, None
            for w in range(N_WAVES):
                for c in range(N_STRIPS):
                    if used[w, c]:
                        continue
                    new = load[w, :, c] + njr[j]
                    key = (new.max(), new.sum(), load[w].sum())
                    if bkey is None or key < bkey:
                        bkey, best = key, (w, c)
            w, c = best
            used[w, c] = True
            load[w, :, c] += njr[j]
            colmap[j] = (q, w, c)
            for (r, kt, n) in sorted(cols[j], key=lambda t: t[1]):
                slots[w][r][c].append((kt, "blk", n))
            rb = int(np.argmin(load[w, :, c]))
            slots[w][rb][c].append((0, "bias", j))
            load[w, rb, c] += 1
        qslots.append(slots)

    # cross-quarter uniform group sizes (pad with zero-weight slots)
    S = np.zeros((N_WAVES, 4, N_STRIPS), np.int64)
    for w in range(N_WAVES):
        for r in range(4):
            for c in range(N_STRIPS):
                S[w, r, c] = max(1, max(len(qslots[q][w][r][c])
                                        for q in range(N_Q)))
    for q in range(N_Q):
        for w in range(N_WAVES):
            for r in range(4):
                for c in range(N_STRIPS):
                    lst = qslots[q][w][r][c]
                    while len(lst) < S[w, r, c]:
                        lst.append((0, "pad", 0))

    # emission order (identical for all quarters): per wave, r-inner
    # round-robin across the 16 sub-arrays; weight cols sequential per r.
    emit = [[] for _ in range(N_WAVES)]   # [(r, c, i, wcol)]
    wcount = [0, 0, 0, 0]
    for w in range(N_WAVES):
        mx = int(S[w].max())
        for i in range(mx):
            for c in range(N_STRIPS):
                for r in range(4):
                    if i < S[w, r, c]:
                        emit[w].append((r, c, i, wcount[r]))
                        wcount[r] += 1
    return {"qslots": qslots, "emit": emit, "S": S, "wcount": wcount,
            "colmap": colmap, "total_mms": int(S.sum())}


# ----------------------------------------------------------------------
# bass program (4-way branched tensor stream)
# ----------------------------------------------------------------------

def _build_program(sched, wcols):
    nc = bacc.Bacc(trn_type="TRN2")

    xT_d = nc.dram_tensor("xT", [128, N_KT * B_PER_CORE], BF16,
                          kind="ExternalInput")
    wK_d = nc.dram_tensor("wK", [128, wcols], BF16, kind="ExternalInput")
    outT_d = nc.dram_tensor("outT", [N_WAVES, 128, 512], BF16,
                            kind="ExternalOutput")

    qslots, emit = sched["qslots"], sched["emit"]

    with tile.TileContext(nc) as tc:
        with (
            tc.tile_pool(name="xp", bufs=1) as xp,
            tc.tile_pool(name="wp", bufs=1) as wp,
            tc.tile_pool(name="cp", bufs=1) as cp,
            tc.tile_pool(name="sp", bufs=2) as sp,
            tc.tile_pool(name="op", bufs=2) as op,
            tc.tile_pool(name="ps", bufs=2, space="PSUM") as ps,
        ):
            quads = [ps.tile([128, 2048], F32, tag="quad", name=f"quad{p}")
                     for p in range(2)]
            ones_t = cp.tile([128, B_PER_CORE], BF16)
            nc.gpsimd.memset(ones_t[:], 1.0)

            xt = xp.tile([128, N_KT * B_PER_CORE], BF16)
            wt = wp.tile([128, wcols], BF16)

            # DMA stream: interleave W (wave-order) and x (k-order) so
            # wave w's weights and early k-tiles arrive first.  Chunk W
            # at the per-wave max weight-column offset across strips.
            wq = [0, 0, 0, 0]
            wave_end = []
            for w in range(N_WAVES):
                for (r, c, i, wcol) in emit[w]:
                    wq[r] = max(wq[r], wcol + 1)
                wave_end.append(max(wq))

            def wchunk(a, b):
                if b > a:
                    nc.sync.dma_start(wt[:, a * BS:b * BS],
                                      wK_d[:, a * BS:b * BS])

            def xchunk(a, b):
                nc.sync.dma_start(xt[:, a * B_PER_CORE:b * B_PER_CORE],
                                  xT_d[:, a * B_PER_CORE:b * B_PER_CORE])

            wchunk(0, wave_end[0])
            xkb = [0, 4, 8, 12, 16, 20, 26, 32]
            for i in range(len(xkb) - 1):
                xchunk(xkb[i], xkb[i + 1])
                if i + 1 < N_WAVES:
                    wchunk(wave_end[i], wave_end[i + 1])
            for w in range(len(xkb), N_WAVES):
                wchunk(wave_end[w - 1], wave_end[w])

            pid = nc.tensor.partition_id()
            qv = pid % 4

            for w in range(N_WAVES):
                quad = quads[w % 2]
                first = {}
                last = {}
                for k, (r, c, i, wcol) in enumerate(emit[w]):
                    if (r, c) not in first:
                        first[(r, c)] = k
                    last[(r, c)] = k
                for q in range(N_Q):
                    with tc.If(qv == q):
                        for k, (r, c, i, wcol) in enumerate(emit[w]):
                            kt, kind, val = qslots[q][w][r][c][i]
                            rhs = (xt[32 * r:32 * r + 32,
                                      kt * B_PER_CORE:(kt + 1) * B_PER_CORE]
                                   if kind == "blk"
                                   else ones_t[32 * r:32 * r + 32, :])
                            nc.tensor.matmul(
                                quad[32 * c:32 * c + 32,
                                     512 * r:512 * r + 512],
                                wt[32 * r:32 * r + 32,
                                   BS * wcol:BS * wcol + BS],
                                rhs,
                                start=(first[(r, c)] == k),
                                stop=(last[(r, c)] == k),
                                skip_group_check=True,
                                tile_position=(32 * r, 32 * c))

                # shared eviction
                st = sp.tile([128, 512], F32, tag="s")
                qview = quad[:, :].rearrange("p (r n) -> p n r", r=4)
                nc.vector.tensor_reduce(st[:], qview, mybir.AxisListType.X,
                                        mybir.AluOpType.add)
                ot = op.tile([128, 512], BF16, tag="o")
                nc.scalar.activation(ot[:], st[:],
                                     mybir.ActivationFunctionType.Relu)
                nc.sync.dma_start(outT_d[w], ot[:])

    nc.compile()
    return nc


# ----------------------------------------------------------------------
# host data prep / result assembly
# ----------------------------------------------------------------------

def _prep_weights(sched, kernel_blocks, bias, wcols):
    """Per-quarter weight arrays [128, wcols] (shared by its 2 cores)."""
    kb = np.asarray(kernel_blocks, np.float32)
    bias = np.asarray(bias, np.float32)
    wKs = []
    for q in range(N_Q):
        wK = np.zeros((128, wcols), ml_dtypes.bfloat16)
        for w in range(N_WAVES):
            for (r, c, i, wcol) in sched["emit"][w]:
                kt, kind, val = sched["qslots"][q][w][r][c][i]
                if kind == "blk":
                    blk = kb[val].astype(ml_dtypes.bfloat16)
                elif kind == "bias":
                    blk = np.zeros((BS, BS), np.float32)
                    blk[0, :] = bias[val * BS:(val + 1) * BS]
                    blk = blk.astype(ml_dtypes.bfloat16)
                else:
                    continue
                wK[32 * r:32 * r + 32, BS * wcol:BS * wcol + BS] = blk
        wKs.append(wK)
    return wKs


def _prep_x(x):
    """Batch-half transposed x: x_sb[p, kt*512+b] = x[h*512+b, 128kt+p]."""
    x_bf = np.asarray(x, np.float32).astype(ml_dtypes.bfloat16)
    xt = np.ascontiguousarray(x_bf.reshape(BATCH, N_KT, 128).transpose(1, 2, 0))
    outs = []
    for h in range(2):
        sl = xt[:, :, h * B_PER_CORE:(h + 1) * B_PER_CORE]
        outs.append(np.ascontiguousarray(
            sl.transpose(1, 0, 2).reshape(128, N_KT * B_PER_CORE)))
    return outs


def _assemble(results, sched):
    out = np.empty((BATCH, N_OUT), np.float32)
    for core in range(N_CORES):
        q, h = core % 4, core // 4
        o = np.asarray(results[core]["outT"], dtype=np.float32)
        for j in range(N_COLS):
            jq, w, c = sched["colmap"][j]
            if jq != q:
                continue
            out[h * B_PER_CORE:(h + 1) * B_PER_CORE,
                j * BS:(j + 1) * BS] = o[w, 32 * c:32 * c + 32, :].T
    return out


def run(x, kernel, bias, ci, co, trace=False):
    ci = np.asarray(ci)
    co = np.asarray(co)
    key = (ci.tobytes(), co.tobytes())
    if _CACHE.get("key") != key:
        sched = _build_schedule(ci, co)
        wcols = BS * max(sched["wcount"])
        wcols = ((wcols + 511) // 512) * 512
        nc = _build_program(sched, wcols)
        _CACHE.update({"key": key, "nc": nc, "sched": sched, "wcols": wcols})
    nc, sched, wcols = _CACHE["nc"], _CACHE["sched"], _CACHE["wcols"]

    wKs = _prep_weights(sched, kernel, bias, wcols)
    xs = _prep_x(x)
    in_maps = [{"xT": xs[c // 4], "wK": wKs[c % 4]} for c in range(N_CORES)]

    last_err = None
    for attempt in range(3):
        try:
            res = run_bass_kernel_spmd(nc, in_maps,
                                       core_ids=list(range(N_CORES)),
                                       trace=trace)
            return _assemble(res.results, sched), res
        except Exception as e:  # transient NRT_EXEC_UNIT_UNRECOVERABLE
            last_err = e
            import time
            time.sleep(2.0)
    raise last_err


def kernel(x, kernel, bias, ci, co):
    out, _ = run(x, kernel, bias, ci, co, trace=False)
    return out


# revision 11
# speedup vs baseline: 2.4086x; 2.4086x over previous
"""Block-sparse matmul kernel for Trainium2 (8 NeuronCores, SPMD).

Problem: out = relu(x @ W_sparse + bias)
  x      [1024, 4096] f32
  kernel [4096, 32, 32] f32   (active 32x32 blocks)
  bias   [4096] f32
  ci, co [4096] int32         (block-row / block-col of each active block)
  out    [1024, 4096] f32

Strategy (v1, dense): scatter blocks into a dense [4096, 4096] weight
matrix on the host, cast x/W to bf16, and run a dense GEMM sharded
2-way over batch x 4-way over output columns (8 cores).  Each core
computes outT = W_slab.T @ x_half.T in [out, batch] orientation so
bias becomes a per-partition scalar for the ScalarE activation
(fused bias + relu straight out of PSUM).  The Bass program is
identical on all cores; only the data differs (SPMD-safe).
"""

import numpy as np
import ml_dtypes

import concourse.bacc as bacc
import concourse.bass as bass
import concourse.mybir as mybir
import concourse.tile as tile
from concourse.bass_utils import run_bass_kernel_spmd

BS = 32
N_IN = 4096
N_OUT = 4096
BATCH = 1024
N_CORES = 8

# sharding grid: 4 output-column quarters x 2 batch halves
CO_SHARDS = 4
B_SHARDS = 2
M_PER_CORE = N_OUT // CO_SHARDS          # 1024 output cols per core
B_PER_CORE = BATCH // B_SHARDS           # 512 batch rows per core
N_MTILES = M_PER_CORE // 128             # 8
N_KTILES = N_IN // 128                   # 32

BF16 = mybir.dt.bfloat16
F32 = mybir.dt.float32

_CACHE = {}


def _build_program():
    """Dense GEMM program, one core's share: outT[m,p,b] = relu(sum_k
    W[k,128m+p] * x[b,k] + bias[128m+p]).  Identical on all cores."""
    nc = bacc.Bacc(trn_type="TRN2")

    xT_d = nc.dram_tensor("xT", [128, N_KTILES * B_PER_CORE], BF16,
                          kind="ExternalInput")
    # wK[p, k*1024 + m*128 + c] = Wdense[128k+p, 128m+c] (per-core slab)
    wK_d = nc.dram_tensor("wK", [128, N_KTILES * N_MTILES * 128], BF16,
                          kind="ExternalInput")
    bias_d = nc.dram_tensor("biasv", [128, N_MTILES], F32,
                            kind="ExternalInput")
    outT_d = nc.dram_tensor("outT", [N_MTILES, 128, B_PER_CORE], F32,
                            kind="ExternalOutput")

    MCOLS = N_MTILES * 128  # 1024 W cols per k-tile

    with tile.TileContext(nc) as tc:
        with (
            tc.tile_pool(name="xp", bufs=1) as xp,
            tc.tile_pool(name="wp", bufs=1) as wp,
            tc.tile_pool(name="bp", bufs=1) as bp,
            tc.tile_pool(name="op", bufs=8) as op,
            tc.tile_pool(name="ps", bufs=8, space="PSUM") as ps,
            tc.tile_pool(name="wu", bufs=1) as wu,
        ):
            accs = [ps.tile([128, B_PER_CORE], F32, tag="acc",
                            name=f"acc{m}")
                    for m in range(N_MTILES)]

            # --- HAM warmup: keep PE busy while the first DMAs land so
            # the real matmul stream starts at K=8/8 (2.4 GHz).
            # memset on gpsimd: it is otherwise idle and its preamble ends
            # earliest, so the PE warmup isn't gated on VectorE's table
            # loads.
            wut = wu.tile([128, 128], BF16)
            nc.gpsimd.memset(wut[:], 0.0)
            for _ in range(16):
                nc.tensor.matmul(accs[N_MTILES - 1][:, 0:128],
                                 wut[:], wut[:], start=True, stop=True)

            # x and W fully resident in SBUF, streamed in k-order chunks.
            # Early chunks are small (fast arrival, bridges the warmup);
            # later chunks use long per-partition rows to amortize the
            # ~0.6us/row DMA overhead.  Sync frees up first after the
            # preamble, so it carries the critical path; gpsimd/scalar
            # carry mid-stream W.
            xt = xp.tile([128, N_KTILES * B_PER_CORE], BF16)
            wt = wp.tile([128, N_KTILES * MCOLS], BF16)

            def xs(a, b):
                return slice(a * B_PER_CORE, b * B_PER_CORE)

            def wsl(a, b):
                return slice(a * MCOLS, b * MCOLS)

            # One sync HWDGE queue sustains ~390 GB/s regardless of chunk
            # size (measured); multiple queues contend and run slower.
            # So: everything on sync, in exact consumption order, finer
            # chunks early so the stream can start ~10us in.
            kb = [0, 1, 2, 3, 4, 5, 6, 8, 10, 12, 16, 20, 24, 28, 32]
            for j in range(len(kb) - 1):
                a, b = kb[j], kb[j + 1]
                nc.sync.dma_start(xt[:, xs(a, b)], xT_d[:, xs(a, b)])
                nc.sync.dma_start(wt[:, wsl(a, b)], wK_d[:, wsl(a, b)])

            bv = bp.tile([128, N_MTILES], F32)
            nc.sync.dma_start(bv[:], bias_d[:])

            def mm(m, k):
                nc.tensor.matmul(
                    accs[m][:],
                    wt[:, k * MCOLS + m * 128: k * MCOLS + (m + 1) * 128],
                    xt[:, k * B_PER_CORE:(k + 1) * B_PER_CORE],
                    start=(k == 0),
                    stop=(k == N_KTILES - 1),
                )

            # Phase 1, k-outer / m-inner: all 8 PSUM banks accumulate
            # concurrently; step k consumes only x[k]+W[k] (384KiB).
            KSPLIT = N_KTILES - 8
            for k in range(KSPLIT):
                for m in range(N_MTILES):
                    mm(m, k)

            # Phase 2, m-outer: finish each m's last k-tiles, then evict
            # while the next m still matmuls.  Evictions alternate
            # ScalarE / VectorE; out-DMAs overlap the stream tail.
            for m in range(N_MTILES):
                for k in range(KSPLIT, N_KTILES):
                    mm(m, k)
                ot = op.tile([128, B_PER_CORE], F32, tag="o")
                if m % 2 == 0:
                    nc.scalar.activation(ot[:], accs[m][:],
                                         mybir.ActivationFunctionType.Relu,
                                         bias=bv[:, m:m + 1])
                else:
                    nc.vector.tensor_scalar(ot[:], accs[m][:],
                                            bv[:, m:m + 1], 0.0,
                                            mybir.AluOpType.add,
                                            mybir.AluOpType.max)
                nc.sync.dma_start(outT_d[m], ot[:])

    nc.compile()
    return nc


def _dense_weight(kernel_blocks, ci, co):
    """Scatter [N_BLK,32,32] blocks into dense [N_IN, N_OUT] (duplicates sum)."""
    nbr, nbc = N_IN // BS, N_OUT // BS
    kb = np.asarray(kernel_blocks, np.float32)
    flat = ci.astype(np.int64) * nbc + co.astype(np.int64)
    wd4 = np.zeros((nbr * nbc, BS, BS), np.float32)
    order = np.argsort(flat, kind="stable")
    sf = flat[order]
    starts = np.flatnonzero(np.r_[True, sf[1:] != sf[:-1]])
    wd4[sf[starts]] = np.add.reduceat(kb[order], starts, axis=0)
    return (wd4.reshape(nbr, nbc, BS, BS).transpose(0, 2, 1, 3)
            .reshape(N_IN, N_OUT))


def _prep_inputs(x, kernel_blocks, bias, ci, co):
    x = np.asarray(x, np.float32)
    bias = np.asarray(bias, np.float32)
    ci = np.asarray(ci)
    co = np.asarray(co)
    wd = _dense_weight(np.asarray(kernel_blocks), ci, co)

    x_bf = x.astype(ml_dtypes.bfloat16)
    wd_bf = wd.astype(ml_dtypes.bfloat16)

    in_maps = []
    for c in range(N_CORES):
        q, h = divmod(c, B_SHARDS)
        xs = x_bf[h * B_PER_CORE:(h + 1) * B_PER_CORE]      # [512, 4096]
        # xT[p, k*512+b] = xs[b, 128k+p]
        xT = np.ascontiguousarray(
            xs.reshape(B_PER_CORE, N_KTILES, 128).transpose(2, 1, 0)
            .reshape(128, N_KTILES * B_PER_CORE))
        ws = wd_bf[:, q * M_PER_CORE:(q + 1) * M_PER_CORE]  # [4096, 1024]
        # wK[p, k*1024 + m*128 + cc] = ws[128k+p, 128m+cc]
        wK = np.ascontiguousarray(
            ws.reshape(N_KTILES, 128, N_MTILES * 128).transpose(1, 0, 2)
            .reshape(128, N_KTILES * N_MTILES * 128))
        bs = bias[q * M_PER_CORE:(q + 1) * M_PER_CORE]
        biasv = np.ascontiguousarray(bs.reshape(N_MTILES, 128).T)
        in_maps.append({"xT": xT, "wK": wK, "biasv": biasv})
    return in_maps


def _assemble(results):
    out = np.empty((BATCH, N_OUT), np.float32)
    for c in range(N_CORES):
        q, h = divmod(c, B_SHARDS)
        o = results[c]["outT"]  # [8, 128, 512] = [m, p, b]
        out[h * B_PER_CORE:(h + 1) * B_PER_CORE,
            q * M_PER_CORE:(q + 1) * M_PER_CORE] = (
            o.transpose(2, 0, 1).reshape(B_PER_CORE, M_PER_CORE))
    return out


def run(x, kernel, bias, ci, co, trace=False):
    if "nc" not in _CACHE:
        _CACHE["nc"] = _build_program()
    nc = _CACHE["nc"]
    in_maps = _prep_inputs(x, kernel, bias, ci, co)
    last_err = None
    for attempt in range(3):
        try:
            res = run_bass_kernel_spmd(nc, in_maps,
                                       core_ids=list(range(N_CORES)),
                                       trace=trace)
            return _assemble(res.results), res
        except Exception as e:  # transient NRT_EXEC_UNIT_UNRECOVERABLE
            last_err = e
            import time
            time.sleep(2.0)
    raise last_err


def kernel(x, kernel, bias, ci, co):
    out, _ = run(x, kernel, bias, ci, co, trace=False)
    return out



# revision 15
# speedup vs baseline: 2.4239x; 1.0063x over previous
"""Block-sparse matmul kernel for Trainium2 (8 NeuronCores, SPMD).

Problem: out = relu(x @ W_sparse + bias)
  x      [1024, 4096] f32
  kernel [4096, 32, 32] f32   (active 32x32 blocks)
  bias   [4096] f32
  ci, co [4096] int32         (block-row / block-col of each active block)
  out    [1024, 4096] f32

Strategy (v1, dense): scatter blocks into a dense [4096, 4096] weight
matrix on the host, cast x/W to bf16, and run a dense GEMM sharded
2-way over batch x 4-way over output columns (8 cores).  Each core
computes outT = W_slab.T @ x_half.T in [out, batch] orientation so
bias becomes a per-partition scalar for the ScalarE activation
(fused bias + relu straight out of PSUM).  The Bass program is
identical on all cores; only the data differs (SPMD-safe).
"""

import numpy as np
import ml_dtypes

import concourse.bacc as bacc
import concourse.bass as bass
import concourse.mybir as mybir
import concourse.tile as tile
from concourse.bass_utils import run_bass_kernel_spmd

BS = 32
N_IN = 4096
N_OUT = 4096
BATCH = 1024
N_CORES = 8

# sharding grid: 4 output-column quarters x 2 batch halves
CO_SHARDS = 4
B_SHARDS = 2
M_PER_CORE = N_OUT // CO_SHARDS          # 1024 output cols per core
B_PER_CORE = BATCH // B_SHARDS           # 512 batch rows per core
N_MTILES = M_PER_CORE // 128             # 8
N_KTILES = N_IN // 128                   # 32

BF16 = mybir.dt.bfloat16
F32 = mybir.dt.float32

_CACHE = {}


def _build_program():
    """Dense GEMM program, one core's share: outT[m,p,b] = relu(sum_k
    W[k,128m+p] * x[b,k] + bias[128m+p]).  Identical on all cores."""
    nc = bacc.Bacc(trn_type="TRN2")

    xT_d = nc.dram_tensor("xT", [128, N_KTILES * B_PER_CORE], BF16,
                          kind="ExternalInput")
    # wK[p, k*1024 + m*128 + c] = Wdense[128k+p, 128m+c] (per-core slab)
    wK_d = nc.dram_tensor("wK", [128, N_KTILES * N_MTILES * 128], BF16,
                          kind="ExternalInput")
    bias_d = nc.dram_tensor("biasv", [128, N_MTILES], F32,
                            kind="ExternalInput")
    outT_d = nc.dram_tensor("outT", [N_MTILES, 128, B_PER_CORE], BF16,
                            kind="ExternalOutput")

    MCOLS = N_MTILES * 128  # 1024 W cols per k-tile

    with tile.TileContext(nc) as tc:
        with (
            tc.tile_pool(name="xp", bufs=1) as xp,
            tc.tile_pool(name="wp", bufs=1) as wp,
            tc.tile_pool(name="bp", bufs=1) as bp,
            tc.tile_pool(name="op", bufs=8) as op,
            tc.tile_pool(name="ps", bufs=8, space="PSUM") as ps,
            tc.tile_pool(name="wu", bufs=1) as wu,
        ):
            accs = [ps.tile([128, B_PER_CORE], F32, tag="acc",
                            name=f"acc{m}")
                    for m in range(N_MTILES)]

            # --- HAM warmup: keep PE busy while the first DMAs land so
            # the real matmul stream starts at K=8/8 (2.4 GHz).
            # memset on gpsimd: it is otherwise idle and its preamble ends
            # earliest, so the PE warmup isn't gated on VectorE's table
            # loads.
            wut = wu.tile([128, 128], BF16)
            nc.gpsimd.memset(wut[:], 0.0)
            for _ in range(40):
                nc.tensor.matmul(accs[N_MTILES - 1][:, 0:128],
                                 wut[:], wut[:], start=True, stop=True)

            # x and W fully resident in SBUF, streamed in k-order chunks.
            # Early chunks are small (fast arrival, bridges the warmup);
            # later chunks use long per-partition rows to amortize the
            # ~0.6us/row DMA overhead.  Sync frees up first after the
            # preamble, so it carries the critical path; gpsimd/scalar
            # carry mid-stream W.
            xt = xp.tile([128, N_KTILES * B_PER_CORE], BF16)
            wt = wp.tile([128, N_KTILES * MCOLS], BF16)

            def xs(a, b):
                return slice(a * B_PER_CORE, b * B_PER_CORE)

            def wsl(a, b):
                return slice(a * MCOLS, b * MCOLS)

            # One sync HWDGE queue sustains ~390 GB/s regardless of chunk
            # size (measured); multiple queues contend and run slower.
            # So: everything on sync, in exact consumption order, finer
            # chunks early so the stream can start ~10us in.
            kb = [0, 1, 2, 3, 4, 5, 6, 8, 10, 12, 16, 20, 24, 28, 32]
            for j in range(len(kb) - 1):
                a, b = kb[j], kb[j + 1]
                nc.sync.dma_start(xt[:, xs(a, b)], xT_d[:, xs(a, b)])
                nc.sync.dma_start(wt[:, wsl(a, b)], wK_d[:, wsl(a, b)])

            bv = bp.tile([128, N_MTILES], F32)
            nc.sync.dma_start(bv[:], bias_d[:])

            def mm(m, k):
                nc.tensor.matmul(
                    accs[m][:],
                    wt[:, k * MCOLS + m * 128: k * MCOLS + (m + 1) * 128],
                    xt[:, k * B_PER_CORE:(k + 1) * B_PER_CORE],
                    start=(k == 0),
                    stop=(k == N_KTILES - 1),
                )

            # Phase 1, k-outer / m-inner: all 8 PSUM banks accumulate
            # concurrently; step k consumes only x[k]+W[k] (384KiB).
            KSPLIT = N_KTILES - 4
            for k in range(KSPLIT):
                for m in range(N_MTILES):
                    mm(m, k)

            # Phase 2, m-outer: finish each m's last k-tiles, then evict
            # while the next m still matmuls.  Evictions alternate
            # ScalarE / VectorE; out-DMAs overlap the stream tail.
            for m in range(N_MTILES):
                for k in range(KSPLIT, N_KTILES):
                    mm(m, k)
                ot = op.tile([128, B_PER_CORE], BF16, tag="o")
                if m % 2 == 1:
                    nc.scalar.activation(ot[:], accs[m][:],
                                         mybir.ActivationFunctionType.Relu,
                                         bias=bv[:, m:m + 1])
                else:
                    nc.vector.tensor_scalar(ot[:], accs[m][:],
                                            bv[:, m:m + 1], 0.0,
                                            mybir.AluOpType.add,
                                            mybir.AluOpType.max)
                nc.sync.dma_start(outT_d[m], ot[:])

    nc.compile()
    return nc


def _dense_weight(kernel_blocks, ci, co):
    """Scatter [N_BLK,32,32] blocks into dense [N_IN, N_OUT] (duplicates sum)."""
    nbr, nbc = N_IN // BS, N_OUT // BS
    kb = np.asarray(kernel_blocks, np.float32)
    flat = ci.astype(np.int64) * nbc + co.astype(np.int64)
    wd4 = np.zeros((nbr * nbc, BS, BS), np.float32)
    order = np.argsort(flat, kind="stable")
    sf = flat[order]
    starts = np.flatnonzero(np.r_[True, sf[1:] != sf[:-1]])
    wd4[sf[starts]] = np.add.reduceat(kb[order], starts, axis=0)
    return (wd4.reshape(nbr, nbc, BS, BS).transpose(0, 2, 1, 3)
            .reshape(N_IN, N_OUT))


def _prep_inputs(x, kernel_blocks, bias, ci, co):
    x = np.asarray(x, np.float32)
    bias = np.asarray(bias, np.float32)
    ci = np.asarray(ci)
    co = np.asarray(co)
    wd = _dense_weight(np.asarray(kernel_blocks), ci, co)

    x_bf = x.astype(ml_dtypes.bfloat16)
    wd_bf = wd.astype(ml_dtypes.bfloat16)

    in_maps = []
    for c in range(N_CORES):
        q, h = divmod(c, B_SHARDS)
        xs = x_bf[h * B_PER_CORE:(h + 1) * B_PER_CORE]      # [512, 4096]
        # xT[p, k*512+b] = xs[b, 128k+p]
        xT = np.ascontiguousarray(
            xs.reshape(B_PER_CORE, N_KTILES, 128).transpose(2, 1, 0)
            .reshape(128, N_KTILES * B_PER_CORE))
        ws = wd_bf[:, q * M_PER_CORE:(q + 1) * M_PER_CORE]  # [4096, 1024]
        # wK[p, k*1024 + m*128 + cc] = ws[128k+p, 128m+cc]
        wK = np.ascontiguousarray(
            ws.reshape(N_KTILES, 128, N_MTILES * 128).transpose(1, 0, 2)
            .reshape(128, N_KTILES * N_MTILES * 128))
        bs = bias[q * M_PER_CORE:(q + 1) * M_PER_CORE]
        biasv = np.ascontiguousarray(bs.reshape(N_MTILES, 128).T)
        in_maps.append({"xT": xT, "wK": wK, "biasv": biasv})
    return in_maps


def _assemble(results):
    out = np.empty((BATCH, N_OUT), np.float32)
    for c in range(N_CORES):
        q, h = divmod(c, B_SHARDS)
        o = results[c]["outT"]  # [8, 128, 512] = [m, p, b]
        out[h * B_PER_CORE:(h + 1) * B_PER_CORE,
            q * M_PER_CORE:(q + 1) * M_PER_CORE] = (
            o.transpose(2, 0, 1).reshape(B_PER_CORE, M_PER_CORE))
    return out


def run(x, kernel, bias, ci, co, trace=False):
    if "nc" not in _CACHE:
        _CACHE["nc"] = _build_program()
    nc = _CACHE["nc"]
    in_maps = _prep_inputs(x, kernel, bias, ci, co)
    last_err = None
    for attempt in range(3):
        try:
            res = run_bass_kernel_spmd(nc, in_maps,
                                       core_ids=list(range(N_CORES)),
                                       trace=trace)
            return _assemble(res.results), res
        except Exception as e:  # transient NRT_EXEC_UNIT_UNRECOVERABLE
            last_err = e
            import time
            time.sleep(2.0)
    raise last_err


def kernel(x, kernel, bias, ci, co):
    out, _ = run(x, kernel, bias, ci, co, trace=False)
    return out



# revision 16
# speedup vs baseline: 2.4274x; 1.0015x over previous
"""Block-sparse matmul kernel for Trainium2 (8 NeuronCores, SPMD).

Problem: out = relu(x @ W_sparse + bias)
  x      [1024, 4096] f32
  kernel [4096, 32, 32] f32   (active 32x32 blocks)
  bias   [4096] f32
  ci, co [4096] int32         (block-row / block-col of each active block)
  out    [1024, 4096] f32

Strategy (v1, dense): scatter blocks into a dense [4096, 4096] weight
matrix on the host, cast x/W to bf16, and run a dense GEMM sharded
2-way over batch x 4-way over output columns (8 cores).  Each core
computes outT = W_slab.T @ x_half.T in [out, batch] orientation so
bias becomes a per-partition scalar for the ScalarE activation
(fused bias + relu straight out of PSUM).  The Bass program is
identical on all cores; only the data differs (SPMD-safe).
"""

import numpy as np
import ml_dtypes

import concourse.bacc as bacc
import concourse.bass as bass
import concourse.mybir as mybir
import concourse.tile as tile
from concourse.bass_utils import run_bass_kernel_spmd

BS = 32
N_IN = 4096
N_OUT = 4096
BATCH = 1024
N_CORES = 8

# sharding grid: 4 output-column quarters x 2 batch halves
CO_SHARDS = 4
B_SHARDS = 2
M_PER_CORE = N_OUT // CO_SHARDS          # 1024 output cols per core
B_PER_CORE = BATCH // B_SHARDS           # 512 batch rows per core
N_MTILES = M_PER_CORE // 128             # 8
N_KTILES = N_IN // 128                   # 32

BF16 = mybir.dt.bfloat16
F32 = mybir.dt.float32

_CACHE = {}


def _build_program():
    """Dense GEMM program, one core's share: outT[m,p,b] = relu(sum_k
    W[k,128m+p] * x[b,k] + bias[128m+p]).  Identical on all cores."""
    nc = bacc.Bacc(trn_type="TRN2")

    xT_d = nc.dram_tensor("xT", [128, N_KTILES * B_PER_CORE], BF16,
                          kind="ExternalInput")
    # wK[p, k*1024 + m*128 + c] = Wdense[128k+p, 128m+c] (per-core slab)
    wK_d = nc.dram_tensor("wK", [128, N_KTILES * N_MTILES * 128], BF16,
                          kind="ExternalInput")
    bias_d = nc.dram_tensor("biasv", [128, N_MTILES], F32,
                            kind="ExternalInput")
    outT_d = nc.dram_tensor("outT", [N_MTILES, 128, B_PER_CORE], BF16,
                            kind="ExternalOutput")

    MCOLS = N_MTILES * 128  # 1024 W cols per k-tile

    with tile.TileContext(nc) as tc:
        with (
            tc.tile_pool(name="xp", bufs=1) as xp,
            tc.tile_pool(name="wp", bufs=1) as wp,
            tc.tile_pool(name="bp", bufs=1) as bp,
            tc.tile_pool(name="op", bufs=8) as op,
            tc.tile_pool(name="ps", bufs=8, space="PSUM") as ps,
            tc.tile_pool(name="wu", bufs=1) as wu,
        ):
            accs = [ps.tile([128, B_PER_CORE], F32, tag="acc",
                            name=f"acc{m}")
                    for m in range(N_MTILES)]

            # --- HAM warmup: keep PE busy while the first DMAs land so
            # the real matmul stream starts at K=8/8 (2.4 GHz).
            # memset on gpsimd: it is otherwise idle and its preamble ends
            # earliest, so the PE warmup isn't gated on VectorE's table
            # loads.
            wut = wu.tile([128, 128], BF16)
            nc.gpsimd.memset(wut[:], 0.0)
            for _ in range(40):
                nc.tensor.matmul(accs[N_MTILES - 1][:, 0:128],
                                 wut[:], wut[:], start=True, stop=True)

            # x and W fully resident in SBUF, streamed in k-order chunks.
            # Early chunks are small (fast arrival, bridges the warmup);
            # later chunks use long per-partition rows to amortize the
            # ~0.6us/row DMA overhead.  Sync frees up first after the
            # preamble, so it carries the critical path; gpsimd/scalar
            # carry mid-stream W.
            xt = xp.tile([128, N_KTILES * B_PER_CORE], BF16)
            wt = wp.tile([128, N_KTILES * MCOLS], BF16)

            def xs(a, b):
                return slice(a * B_PER_CORE, b * B_PER_CORE)

            def wsl(a, b):
                return slice(a * MCOLS, b * MCOLS)

            # One sync HWDGE queue sustains ~390 GB/s regardless of chunk
            # size (measured); multiple queues contend and run slower.
            # So: everything on sync, in exact consumption order, finer
            # chunks early so the stream can start ~10us in.
            kb = [0, 1, 2, 3, 4, 5, 6, 8, 10, 12, 16, 20, 24, 28, 32]
            for j in range(len(kb) - 1):
                a, b = kb[j], kb[j + 1]
                nc.sync.dma_start(xt[:, xs(a, b)], xT_d[:, xs(a, b)])
                nc.sync.dma_start(wt[:, wsl(a, b)], wK_d[:, wsl(a, b)])

            bv = bp.tile([128, N_MTILES], F32)
            nc.sync.dma_start(bv[:], bias_d[:])

            def mm(m, k):
                nc.tensor.matmul(
                    accs[m][:],
                    wt[:, k * MCOLS + m * 128: k * MCOLS + (m + 1) * 128],
                    xt[:, k * B_PER_CORE:(k + 1) * B_PER_CORE],
                    start=(k == 0),
                    stop=(k == N_KTILES - 1),
                )

            # Phase 1, k-outer / m-inner: all 8 PSUM banks accumulate
            # concurrently; step k consumes only x[k]+W[k] (384KiB).
            KSPLIT = N_KTILES - 8
            for k in range(KSPLIT):
                for m in range(N_MTILES):
                    mm(m, k)

            # Phase 2, m-outer: finish each m's last k-tiles, then evict
            # while the next m still matmuls.  Evictions alternate
            # ScalarE / VectorE; out-DMAs overlap the stream tail.
            for m in range(N_MTILES):
                for k in range(KSPLIT, N_KTILES):
                    mm(m, k)
                ot = op.tile([128, B_PER_CORE], BF16, tag="o")
                if m % 2 == 1:
                    nc.scalar.activation(ot[:], accs[m][:],
                                         mybir.ActivationFunctionType.Relu,
                                         bias=bv[:, m:m + 1])
                else:
                    nc.vector.tensor_scalar(ot[:], accs[m][:],
                                            bv[:, m:m + 1], 0.0,
                                            mybir.AluOpType.add,
                                            mybir.AluOpType.max)
                nc.sync.dma_start(outT_d[m], ot[:])

    nc.compile()
    return nc


def _dense_weight(kernel_blocks, ci, co):
    """Scatter [N_BLK,32,32] blocks into dense [N_IN, N_OUT] (duplicates sum)."""
    nbr, nbc = N_IN // BS, N_OUT // BS
    kb = np.asarray(kernel_blocks, np.float32)
    flat = ci.astype(np.int64) * nbc + co.astype(np.int64)
    wd4 = np.zeros((nbr * nbc, BS, BS), np.float32)
    order = np.argsort(flat, kind="stable")
    sf = flat[order]
    starts = np.flatnonzero(np.r_[True, sf[1:] != sf[:-1]])
    wd4[sf[starts]] = np.add.reduceat(kb[order], starts, axis=0)
    return (wd4.reshape(nbr, nbc, BS, BS).transpose(0, 2, 1, 3)
            .reshape(N_IN, N_OUT))


def _prep_inputs(x, kernel_blocks, bias, ci, co):
    x = np.asarray(x, np.float32)
    bias = np.asarray(bias, np.float32)
    ci = np.asarray(ci)
    co = np.asarray(co)
    wd = _dense_weight(np.asarray(kernel_blocks), ci, co)

    x_bf = x.astype(ml_dtypes.bfloat16)
    wd_bf = wd.astype(ml_dtypes.bfloat16)

    in_maps = []
    for c in range(N_CORES):
        q, h = divmod(c, B_SHARDS)
        xs = x_bf[h * B_PER_CORE:(h + 1) * B_PER_CORE]      # [512, 4096]
        # xT[p, k*512+b] = xs[b, 128k+p]
        xT = np.ascontiguousarray(
            xs.reshape(B_PER_CORE, N_KTILES, 128).transpose(2, 1, 0)
            .reshape(128, N_KTILES * B_PER_CORE))
        ws = wd_bf[:, q * M_PER_CORE:(q + 1) * M_PER_CORE]  # [4096, 1024]
        # wK[p, k*1024 + m*128 + cc] = ws[128k+p, 128m+cc]
        wK = np.ascontiguousarray(
            ws.reshape(N_KTILES, 128, N_MTILES * 128).transpose(1, 0, 2)
            .reshape(128, N_KTILES * N_MTILES * 128))
        bs = bias[q * M_PER_CORE:(q + 1) * M_PER_CORE]
        biasv = np.ascontiguousarray(bs.reshape(N_MTILES, 128).T)
        in_maps.append({"xT": xT, "wK": wK, "biasv": biasv})
    return in_maps


def _assemble(results):
    out = np.empty((BATCH, N_OUT), np.float32)
    for c in range(N_CORES):
        q, h = divmod(c, B_SHARDS)
        o = results[c]["outT"]  # [8, 128, 512] = [m, p, b]
        out[h * B_PER_CORE:(h + 1) * B_PER_CORE,
            q * M_PER_CORE:(q + 1) * M_PER_CORE] = (
            o.transpose(2, 0, 1).reshape(B_PER_CORE, M_PER_CORE))
    return out


def run(x, kernel, bias, ci, co, trace=False):
    if "nc" not in _CACHE:
        _CACHE["nc"] = _build_program()
    nc = _CACHE["nc"]
    in_maps = _prep_inputs(x, kernel, bias, ci, co)
    last_err = None
    for attempt in range(3):
        try:
            res = run_bass_kernel_spmd(nc, in_maps,
                                       core_ids=list(range(N_CORES)),
                                       trace=trace)
            return _assemble(res.results), res
        except Exception as e:  # transient NRT_EXEC_UNIT_UNRECOVERABLE
            last_err = e
            import time
            time.sleep(2.0)
    raise last_err


def kernel(x, kernel, bias, ci, co):
    out, _ = run(x, kernel, bias, ci, co, trace=False)
    return out

